# revision 26
# baseline (speedup 1.0000x reference)
"""Bass kernel for nn_Attention_80393197847209 on trn2.

Strategy: batch-parallel over the 8 NeuronCores (B=8, one batch element per
core). Stage-1 matmuls run as float32r. The big wq2/wk2 (4608x4608)
projections run as fp8 e4m3 DoubleRow matmuls (2 k-tiles per instruction,
0.5 cycles/row) from host-prepacked, x64-scaled fp8 weights; the 64*64
score scale is folded into the softmax EXP scale. y stays resident in
SBUF as bf16 for the final patt matmul; its transpose is kept as fp8
pairs for the projections.
"""
import math
from contextlib import ExitStack

import ml_dtypes
import numpy as np

import concourse.bacc as bacc
import concourse.mybir as mybir
import concourse.tile as tile
from concourse.masks import make_identity

P = 128
CL, QL, H, E2 = 512, 64, 768, 4608
CT_N = CL // P   # 4 c tiles
HT = H // P      # 6 h tiles
ET = E2 // P     # 36 e tiles
ET2 = ET // 2    # 18 e pair-tiles
HD = 192         # head dim for both mha blocks
NHEAD1, NHEAD2 = 4, 24
NPAIR = NHEAD2 // 2  # head pairs in stage 2
ISQ = 1.0 / math.sqrt(HD)
WSCALE = 64.0        # fp8 weight prescale (both sides) in stage 2
ISQ2 = ISQ / (WSCALE * WSCALE)
NEG = -1e30
EPS = 1e-5

f32 = mybir.dt.float32
f32r = mybir.dt.float32r
bf16 = mybir.dt.bfloat16
f8 = mybir.dt.float8e4
u8 = mybir.dt.uint8
DR = mybir.MatmulPerfMode.DoubleRow
EXP = mybir.ActivationFunctionType.Exp
SQRT = mybir.ActivationFunctionType.Sqrt
AX = mybir.AxisListType.X
MAX = mybir.AluOpType.max
MULT = mybir.AluOpType.mult
ADD = mybir.AluOpType.add
SUB = mybir.AluOpType.subtract

# x slice offsets: [c | a | c*a | c*b | scoat3 | acoat]
XO_C, XO_A, XO_CA, XO_CB, XO_S3, XO_AC = (i * H for i in range(6))


def _masked_softmax(nc, pool, src, out, m_b, nm_b, p, f, tag):
    """out = softmax over free dim of (src + nm), max-subtracted.

    Add-only masking: nm is -1e30 on masked entries, 0 elsewhere; since
    |src| << 1e30 this matches the reference mask exactly for any row
    with at least one unmasked entry."""
    l = pool.tile([p, f], f32, tag=f"l_{tag}", name=f"l_{tag}")
    nc.vector.tensor_add(l, src, nm_b[0:p, 0:f])
    mx = pool.tile([p, 1], f32, tag=f"mx_{tag}", name=f"mx_{tag}")
    nc.vector.tensor_reduce(mx, l, axis=AX, op=MAX, negate=True)
    e = pool.tile([p, f], f32, tag=f"e_{tag}", name=f"e_{tag}")
    sm = pool.tile([p, 1], f32, tag=f"sm_{tag}", name=f"sm_{tag}")
    nc.scalar.activation(e, l, EXP, bias=mx, scale=1.0, accum_out=sm)
    r = pool.tile([p, 1], f32, tag=f"r_{tag}", name=f"r_{tag}")
    nc.vector.reciprocal(r, sm)
    nc.vector.tensor_scalar_mul(out, e, r)


def build(num_devices=8, debug=False, trivial_gb=False):
    nc = bacc.Bacc("TRN2", target_bir_lowering=False, debug=False,
                   num_devices=num_devices)

    # ---- DRAM I/O ----
    d_c = nc.dram_tensor("c", (CL, H), f32r, kind="ExternalInput")
    d_q = nc.dram_tensor("q", (QL, H), f32r, kind="ExternalInput")
    d_cT = nc.dram_tensor("cT", (H, CL), f32r, kind="ExternalInput")
    d_qT = nc.dram_tensor("qT", (H, QL), f32r, kind="ExternalInput")
    d_cw = nc.dram_tensor("cw2", (H, 2), f32r, kind="ExternalInput")
    d_qw = nc.dram_tensor("qw2", (H, 2), f32r, kind="ExternalInput")
    d_cqw = nc.dram_tensor("cq_weight", (H,), f32, kind="ExternalInput")
    d_bias = nc.dram_tensor("bias", (1, 1), f32, kind="ExternalInput")
    d_wq1t = nc.dram_tensor("wq1t", (H, H), f32r, kind="ExternalInput")
    d_wk1t = nc.dram_tensor("wk1t", (H, H), f32r, kind="ExternalInput")
    d_bq1 = nc.dram_tensor("bq1", (H,), f32, kind="ExternalInput")
    d_bk1 = nc.dram_tensor("bk1", (H,), f32, kind="ExternalInput")
    d_gamma = nc.dram_tensor("gamma", (E2,), f32, kind="ExternalInput")
    d_beta = nc.dram_tensor("beta", (E2,), f32, kind="ExternalInput")
    # packed fp8 e4m3 (uint8 carrier) stage-2 weights, x64 scaled:
    # [pair(12), t(18), p(128)] rows x [i(2), m(384)] cols
    d_wq28 = nc.dram_tensor("wq28", (NPAIR * ET2 * P, 768), u8,
                            kind="ExternalInput")
    d_wk28 = nc.dram_tensor("wk28", (NPAIR * ET2 * P, 768), u8,
                            kind="ExternalInput")
    d_bq2 = nc.dram_tensor("bq2", (E2,), f32, kind="ExternalInput")
    d_bk2 = nc.dram_tensor("bk2", (E2,), f32, kind="ExternalInput")
    d_ident = nc.dram_tensor("identity", (P, P), f32, kind="ExternalInput")
    d_qm = nc.dram_tensor("qm", (QL,), f32, kind="ExternalInput")
    d_nqm = nc.dram_tensor("nqm", (QL,), f32, kind="ExternalInput")
    d_cm = nc.dram_tensor("cm", (CL,), f32, kind="ExternalInput")
    d_ncm = nc.dram_tensor("ncm", (CL,), f32, kind="ExternalInput")
    d_out = nc.dram_tensor("out", (CL, E2), f32, kind="ExternalOutput")

    dbg = {}
    if debug:
        for name, shape in [("dbg_s", (QL, CL)), ("dbg_s2m", (QL, CL)),
                            ("dbg_scoat", (CL, QL)), ("dbg_x", (CL, E2)),
                            ("dbg_y", (CL, E2)), ("dbg_ss", (CL, CL)),
                            ("dbg_qh2t", (E2, CL))]:
            dbg[name] = nc.dram_tensor(name, shape, f32, kind="ExternalOutput")

    with tile.TileContext(nc) as tc, ExitStack() as es:
        const = es.enter_context(tc.tile_pool(name="const", bufs=1))
        dram = es.enter_context(tc.tile_pool(name="dram", bufs=1,
                                             space="DRAM"))

        # ---- constants / masks ----
        ident = const.tile([P, P], f32, tag="ident", name="ident")
        nc.sync.dma_start(out=ident, in_=d_ident[:, :])
        ident16 = const.tile([P, P], bf16, tag="ident16", name="ident16")
        nc.vector.tensor_copy(ident16, ident)
        cwT = const.tile([P, HT, 2], f32r, tag="cwT", name="cwT")
        nc.sync.dma_start(out=cwT,
                          in_=d_cw.ap().rearrange("(t p) k -> p t k", p=P))
        qwT = const.tile([P, HT, 2], f32r, tag="qwT", name="qwT")
        nc.sync.dma_start(out=qwT,
                          in_=d_qw.ap().rearrange("(t p) k -> p t k", p=P))
        cqwT = const.tile([P, HT], f32, tag="cqwT", name="cqwT")
        nc.sync.dma_start(out=cqwT,
                          in_=d_cqw.ap().rearrange("(t p) -> p t", p=P))
        bq1T = const.tile([P, HT], f32, tag="bq1T", name="bq1T")
        nc.sync.dma_start(out=bq1T,
                          in_=d_bq1.ap().rearrange("(t p) -> p t", p=P))
        bk1T = const.tile([P, HT], f32, tag="bk1T", name="bk1T")
        nc.sync.dma_start(out=bk1T,
                          in_=d_bk1.ap().rearrange("(t p) -> p t", p=P))
        bias_sb = const.tile([1, 1], f32, tag="bias", name="bias")
        nc.sync.dma_start(out=bias_sb, in_=d_bias[:, :])
        eps_sb = const.tile([P, 1], f32, tag="eps", name="eps")
        nc.vector.memset(eps_sb, EPS)

        def pe_T(in_ap, pool):
            """PE transpose: returns PSUM AP [f, p] = in_ap.T (f32)."""
            p = in_ap.partition_size()
            f = in_ap.free_size()
            pst = pool.tile([P, 2, P], f32, tag="tr", name="tr")
            out = pst[0:f, 0, 0:p]
            nc.tensor.transpose(out, in_ap, ident[0:p, 0:p])
            return out

        ytp_es = ExitStack()

        x_re = []
        wst = es.enter_context(tc.tile_pool(name="wst", bufs=2))

        # ================= stage 1 =================
        s1es = ExitStack()
        s1bes = ExitStack()
        with s1bes, s1es:
            s1b = s1bes.enter_context(tc.tile_pool(name="s1b", bufs=1))
            bigp = s1bes.enter_context(
                tc.tile_pool(name="bigp", bufs=1, space="PSUM"))
            trp = s1bes.enter_context(
                tc.tile_pool(name="trp", bufs=2, space="PSUM"))
            s1a = s1es.enter_context(
                tc.tile_pool(name="s1a", bufs=1, side="right"))
            smallp = s1es.enter_context(
                tc.tile_pool(name="smallp", bufs=2, space="PSUM"))
            w1es = ExitStack()
            w1p = w1es.enter_context(
                tc.tile_pool(name="w1p", bufs=1, side="right"))

            crows = []
            for i in range(CT_N):
                t = s1b.tile([P, H], f32r, tag=f"crows{i}", name=f"crows{i}")
                nc.sync.dma_start(out=t, in_=d_c[i * P:(i + 1) * P, :])
                crows.append(t)
            qrows = s1b.tile([QL, H], f32r, tag="qrows", name="qrows")
            nc.sync.dma_start(out=qrows, in_=d_q[:, :])


            wq1t_sb, wk1t_sb = [], []
            for j in range(HT):
                t = w1p.tile([P, H], f32r, tag=f"wq1t{j}", name=f"wq1t{j}")
                nc.sync.dma_start(out=t, in_=d_wq1t[j * P:(j + 1) * P, :])
                wq1t_sb.append(t)
                t = w1p.tile([P, H], f32r, tag=f"wk1t{j}", name=f"wk1t{j}")
                nc.sync.dma_start(out=t, in_=d_wk1t[j * P:(j + 1) * P, :])
                wk1t_sb.append(t)

            qm_b = const.tile([P, QL], f32, tag="qm_b", name="qm_b")
            nc.sync.dma_start(out=qm_b, in_=d_qm.ap().partition_broadcast(P))
            nqm_b = const.tile([P, QL], f32, tag="nqm_b", name="nqm_b")
            nc.sync.dma_start(out=nqm_b, in_=d_nqm.ap().partition_broadcast(P))
            cm_b64 = const.tile([QL, CL], f32, tag="cm_b64", name="cm_b64")
            nc.sync.dma_start(out=cm_b64, in_=d_cm.ap().partition_broadcast(QL))
            ncm_b64 = const.tile([QL, CL], f32, tag="ncm_b64", name="ncm_b64")
            nc.sync.dma_start(out=ncm_b64, in_=d_ncm.ap().partition_broadcast(QL))

            # prefetch the pair-0 stage-2 weight slabs during stage 1 so
            # the first projections start without a DMA bubble (emitted
            # after the stage-1-critical loads so it doesn't delay them)
            w0 = {}
            for side, dw in (("q", d_wq28), ("k", d_wk28)):
                t = wst.tile([P, ET2, 768], u8, tag="wchunk",
                             name="wchunk")
                nc.sync.dma_start(
                    out=t,
                    in_=dw.ap()[0:ET2 * P, :].rearrange("(t p) f -> p t f",
                                                        p=P))
                w0[side] = t
            # CT[j]: [128h, 512c], QT[j]: [128h, 64q] (host-transposed)
            ct, qt = [], []
            for j in range(HT):
                tj = s1a.tile([P, CL], f32r, tag=f"ct{j}", name=f"ct{j}")
                nc.sync.dma_start(out=tj, in_=d_cT[j * P:(j + 1) * P, :])
                ct.append(tj)
                qj = s1a.tile([P, QL], f32r, tag=f"qt{j}", name=f"qt{j}")
                nc.sync.dma_start(out=qj, in_=d_qT[j * P:(j + 1) * P, :])
                qt.append(qj)

            # mha1 projections early (frees wq1t/wk1t)
            qh1T, kh1T = [], []
            for e in range(HT):
                ps = smallp.tile([P, CL], f32, tag="smA", name="qh1")
                for j in range(HT):
                    nc.tensor.matmul(ps, wq1t_sb[j][:, e * P:(e + 1) * P],
                                     ct[j], start=(j == 0),
                                     stop=(j == HT - 1))
                t = s1a.tile([P, CL], f32r, tag=f"qh1T{e}", name=f"qh1T{e}")
                nc.scalar.add(t, ps, bq1T[:, e:e + 1])
                qh1T.append(t)
                ps = smallp.tile([P, QL], f32, tag="smB", name="kh1")
                for j in range(HT):
                    nc.tensor.matmul(ps, wk1t_sb[j][:, e * P:(e + 1) * P],
                                     qt[j], start=(j == 0),
                                     stop=(j == HT - 1))
                t = s1a.tile([P, QL], f32r, tag=f"kh1T{e}", name=f"kh1T{e}")
                nc.scalar.add(t, ps, bk1T[:, e:e + 1])
                kh1T.append(t)
            w1es.close()

            # CWT[j] = CT[j] * cqw[j]
            cwt = []
            for j in range(HT):
                tj = s1a.tile([P, CL], f32r, tag=f"cwt{j}", name=f"cwt{j}")
                nc.scalar.activation(tj, ct[j].bitcast(f32),
                                     mybir.ActivationFunctionType.Identity,
                                     scale=cqwT[:, j:j + 1])
                cwt.append(tj)

            # ---- s matrices ----
            s0_ps = smallp.tile([2, CL], f32, tag="smA", name="s0")
            for j in range(HT):
                nc.tensor.matmul(s0_ps, cwT[:, j, :], ct[j],
                                 start=(j == 0), stop=(j == HT - 1))
            s1_ps = smallp.tile([2, QL], f32, tag="smB", name="s1c")
            for j in range(HT):
                nc.tensor.matmul(s1_ps, qwT[:, j, :], qt[j],
                                 start=(j == 0), stop=(j == HT - 1))

            # augmented K=1 operands: sT += s1row x ones + ones x (s0+bias)
            s1row = s1a.tile([1, QL], f32r, tag="s1row", name="s1row")
            nc.vector.tensor_copy(s1row, s1_ps[0:1, :])
            ones64 = s1a.tile([1, QL], f32r, tag="ones64", name="ones64")
            nc.vector.memset(ones64.bitcast(f32), 1.0)
            s0brow = s1a.tile([1, CL], f32r, tag="s0brow", name="s0brow")
            nc.vector.tensor_scalar_add(s0brow, s0_ps[0:1, :],
                                        bias_sb[0:1, :])
            ones512 = s1a.tile([1, CL], f32r, tag="ones512", name="ones512")
            nc.vector.memset(ones512.bitcast(f32), 1.0)

            sT_ps = smallp.tile([QL, CL], f32, tag="smA", name="sT")
            for j in range(HT):
                nc.tensor.matmul(sT_ps, qt[j], cwt[j], start=(j == 0),
                                 stop=False)
            nc.tensor.matmul(sT_ps, s1row, ones512, start=False, stop=False)
            nc.tensor.matmul(sT_ps, ones64, s0brow, start=False, stop=True)
            s_qc = s1a.tile([QL, CL], f32, tag="s_qc", name="s_qc")
            nc.vector.tensor_copy(s_qc, sT_ps)
            if dbg:
                nc.sync.dma_start(out=dbg["dbg_s"][:, :], in_=s_qc)

            # s2m in [q, c]
            s2m_qc = s1a.tile([QL, CL], f32r, tag="s2m_qc", name="s2m_qc")
            _masked_softmax(nc, s1a, s_qc, s2m_qc, cm_b64, ncm_b64, QL, CL,
                            "s2m")
            if dbg:
                nc.sync.dma_start(out=dbg["dbg_s2m"][:, :],
                                  in_=s2m_qc.bitcast(f32))

            # s1m in [c, q]
            s1m_cq = []
            for i in range(CT_N):
                sc = s1a.tile([P, QL], f32, tag=f"s_cq{i}", name=f"s_cq{i}")
                nc.vector.tensor_copy(sc, pe_T(s_qc[:, i * P:(i + 1) * P],
                                               trp))
                sm = s1a.tile([P, QL], f32, tag=f"s1m_cq{i}", name=f"s1m_cq{i}")
                _masked_softmax(nc, s1a, sc, sm, qm_b, nqm_b, P, QL,
                                f"s1m{i}")
                s1m_cq.append(sm)
            s1mT = s1b.tile([QL, CL], f32r, tag="s1mT", name="s1mT")
            for i in range(CT_N):
                nc.vector.tensor_copy(s1mT[:, i * P:(i + 1) * P],
                                      pe_T(s1m_cq[i], trp))

            # tT[d] [128d, 512c]
            tT_sb = []
            for d in range(CT_N):
                ps = smallp.tile([P, CL], f32, tag="smA", name="tT")
                nc.tensor.matmul(ps, s2m_qc[:, d * P:(d + 1) * P], s1mT,
                                 start=True, stop=True)
                t = s1b.tile([P, CL], f32r, tag=f"tT{d}", name=f"tT{d}")
                nc.vector.tensor_copy(t, ps)
                tT_sb.append(t)

            # ---- mha1 scores + scoat ----
            def _sub(tiles, src_j, lo, width, tag):
                t = s1a.tile([64, width], f32r, tag=tag)
                nc.vector.tensor_copy(t,
                                      tiles[src_j][lo:lo + 64, :].bitcast(f32))
                return t

            q_sub = {0: _sub(qh1T, 1, 0, CL, "qs0"),
                     1: _sub(qh1T, 1, 64, CL, "qs1"),
                     2: _sub(qh1T, 4, 0, CL, "qs2"),
                     3: _sub(qh1T, 4, 64, CL, "qs3")}
            k_sub = {0: _sub(kh1T, 1, 0, QL, "ks0"),
                     1: _sub(kh1T, 1, 64, QL, "ks1"),
                     2: _sub(kh1T, 4, 0, QL, "ks2"),
                     3: _sub(kh1T, 4, 64, QL, "ks3")}
            head_ops = {
                0: [(qh1T[0], kh1T[0]), (q_sub[0], k_sub[0])],
                1: [(q_sub[1], k_sub[1]), (qh1T[2], kh1T[2])],
                2: [(qh1T[3], kh1T[3]), (q_sub[2], k_sub[2])],
                3: [(q_sub[3], k_sub[3]), (qh1T[5], kh1T[5])],
            }

            scoat_cq = [s1a.tile([P, QL], f32, tag=f"scoat{i}", name=f"scoat{i}")
                        for i in range(CT_N)]
            for h in range(NHEAD1):
                for i in range(CT_N):
                    ps = smallp.tile([P, QL], f32, tag="smB", name="sc1")
                    ops = head_ops[h]
                    for ki, (ql, kr) in enumerate(ops):
                        nc.tensor.matmul(ps, ql[:, i * P:(i + 1) * P], kr,
                                         start=(ki == 0),
                                         stop=(ki == len(ops) - 1))
                    u = f"{h}_{i}"
                    # no max-subtract: mha1 logits are tiny
                    e_sb = s1a.tile([P, QL], f32, tag=f"e1{u}", name=f"e1{u}")
                    ssum = s1a.tile([P, 1], f32, tag=f"ssum1{u}", name=f"ssum1{u}")
                    nc.scalar.activation(e_sb, ps, EXP, scale=ISQ,
                                         accum_out=ssum)
                    r = s1a.tile([P, 1], f32, tag=f"r1{u}", name=f"r1{u}")
                    nc.vector.reciprocal(r, ssum)
                    r4 = s1a.tile([P, 1], f32, tag=f"r41{u}", name=f"r41{u}")
                    nc.vector.tensor_scalar_mul(r4, r, 1.0 / NHEAD1)
                    if h == 0:
                        nc.vector.tensor_scalar_mul(scoat_cq[i], e_sb, r4)
                    else:
                        nc.vector.scalar_tensor_tensor(
                            scoat_cq[i], in0=e_sb, scalar=r4,
                            in1=scoat_cq[i], op0=MULT, op1=ADD)
            if dbg:
                for i in range(CT_N):
                    nc.sync.dma_start(
                        out=dbg["dbg_scoat"][i * P:(i + 1) * P, :],
                        in_=scoat_cq[i])

            # scoat1 -> scoat1T (f32r)
            scoat1T = s1b.tile([QL, CL], f32r, tag="scoat1T", name="scoat1T")
            for i in range(CT_N):
                sm = s1a.tile([P, QL], f32, tag=f"scoat1_{i}", name=f"scoat1_{i}")
                _masked_softmax(nc, s1a, scoat_cq[i], sm, qm_b, nqm_b, P, QL,
                                f"sc1_{i}")
                nc.vector.tensor_copy(scoat1T[:, i * P:(i + 1) * P],
                                      pe_T(sm, trp))

            # scoatT -> scoat2_qc -> scoat2_cq (f32r)
            scoatT = s1a.tile([QL, CL], f32, tag="scoatT", name="scoatT")
            for i in range(CT_N):
                nc.vector.tensor_copy(scoatT[:, i * P:(i + 1) * P],
                                      pe_T(scoat_cq[i], trp))
            scoat2_qc = s1a.tile([QL, CL], f32, tag="scoat2_qc", name="scoat2_qc")
            _masked_softmax(nc, s1a, scoatT, scoat2_qc, cm_b64, ncm_b64,
                            QL, CL, "sc2")
            scoat2_cq = []
            for i in range(CT_N):
                t = s1a.tile([P, QL], f32r, tag=f"scoat2_cq{i}", name=f"scoat2_cq{i}")
                nc.vector.tensor_copy(t,
                                      pe_T(scoat2_qc[:, i * P:(i + 1) * P],
                                           trp))
                scoat2_cq.append(t)

            # bcoat [64q, 768h]
            bc_ps = bigp.tile([QL, H], f32, tag="big768", name="big768")
            for i in range(CT_N):
                nc.tensor.matmul(bc_ps[:, 0:512], scoat2_cq[i],
                                 crows[i][:, 0:512],
                                 start=(i == 0), stop=(i == CT_N - 1))
            for i in range(CT_N):
                nc.tensor.matmul(bc_ps[:, 512:H], scoat2_cq[i],
                                 crows[i][:, 512:H],
                                 start=(i == 0), stop=(i == CT_N - 1))
            bcoat = s1b.tile([QL, H], f32r, tag="bcoat", name="bcoat")
            nc.vector.tensor_copy(bcoat, bc_ps)
            s1es.close()  # free s1a pool, smallp
            trp2 = s1bes.enter_context(
                tc.tile_pool(name="trp2", bufs=4, space="PSUM"))

            # x kept resident through phase 7 (no DRAM round-trip)
            xres = es.enter_context(tc.tile_pool(name="xres", bufs=1,
                                                 side="right"))
            # y kept resident for phase 7 (fp8 pairs, beta included) +
            # fp8 transposed pairs (gamma only; beta folded into the host
            # stage-2 bias). Created late so the early stage-1 pools
            # (w1p etc.) can use the space first.
            yfinp = es.enter_context(tc.tile_pool(name="yfin", bufs=1,
                                                  side="right"))
            ytp = ytp_es.enter_context(tc.tile_pool(name="ytp", bufs=1,
                                                    side="right"))
            y_bf = [yfinp.tile([P, E2], bf16, tag=f"ybf{i}",
                               name=f"ybf{i}") for i in range(CT_N)]
            y8p = [ytp.tile([P, 2, CL], f8, tag=f"y8p{t}",
                            name=f"y8p{t}") for t in range(ET2)]

            # ---- per-c-tile x assembly + LN + park ----
            ysb_pool = s1bes.enter_context(tc.tile_pool(name="ysb", bufs=2))
            gb_pool = s1bes.enter_context(tc.tile_pool(name="gb", bufs=1))
            scr_pool = s1bes.enter_context(tc.tile_pool(name="scr", bufs=1))
            if not trivial_gb:
                gamma_b = gb_pool.tile([P, E2], f32, tag="gamma_b",
                                       name="gamma_b")
                nc.sync.dma_start(out=gamma_b,
                                  in_=d_gamma.ap().partition_broadcast(P))
                beta_b = gb_pool.tile([P, E2], f32, tag="beta_b",
                                      name="beta_b")
                nc.sync.dma_start(out=beta_b,
                                  in_=d_beta.ap().partition_broadcast(P))

            ydt = bf16 if trivial_gb else f32
            yid = ident16 if trivial_gb else ident

            def _y8p_write(pi, py):
                for t in range(ET2):
                    ytr = trp2.tile([P, 2, P], ydt, tag="tr2", name="tr2")
                    for k in range(2):
                        nc.tensor.transpose(
                            ytr[:, k, :],
                            py[:, (2 * t + k) * P:(2 * t + k + 1) * P],
                            yid)
                    nc.vector.tensor_copy(
                        y8p[t][:, :, pi * P:(pi + 1) * P], ytr)

            pending_y = []
            for i in range(CT_N):
                x_i = xres.tile([P, E2], f32, tag=f"x{i}", name=f"x{i}")
                x_re.append(x_i)
                nc.scalar.copy(x_i[:, XO_C:XO_C + H], crows[i].bitcast(f32))
                a_ps = bigp.tile([P, H], f32, tag="big768", name="big768")
                nc.tensor.matmul(a_ps[:, 0:512], s1mT[:, i * P:(i + 1) * P],
                                 qrows[:, 0:512], start=True, stop=True)
                nc.tensor.matmul(a_ps[:, 512:H], s1mT[:, i * P:(i + 1) * P],
                                 qrows[:, 512:H], start=True, stop=True)
                nc.scalar.copy(x_i[:, XO_A:XO_A + H], a_ps)
                nc.gpsimd.tensor_mul(x_i[:, XO_CA:XO_CA + H],
                                      crows[i].bitcast(f32),
                                      x_i[:, XO_A:XO_A + H])
                b_ps = bigp.tile([P, H], f32, tag="big768", name="big768")
                for d in range(CT_N):
                    nc.tensor.matmul(b_ps[:, 0:512],
                                     tT_sb[d][:, i * P:(i + 1) * P],
                                     crows[d][:, 0:512],
                                     start=(d == 0), stop=(d == CT_N - 1))
                for d in range(CT_N):
                    nc.tensor.matmul(b_ps[:, 512:H],
                                     tT_sb[d][:, i * P:(i + 1) * P],
                                     crows[d][:, 512:H],
                                     start=(d == 0), stop=(d == CT_N - 1))
                b_sb = scr_pool.tile([P, H], f32, tag="b_sb", name="b_sb")
                nc.scalar.copy(b_sb, b_ps)
                nc.gpsimd.tensor_mul(x_i[:, XO_CB:XO_CB + H],
                                       crows[i].bitcast(f32), b_sb)
                s3_ps = bigp.tile([P, H], f32, tag="big768", name="big768")
                nc.tensor.matmul(s3_ps[:, 0:512],
                                 scoat1T[:, i * P:(i + 1) * P],
                                 bcoat[:, 0:512], start=True, stop=True)
                nc.tensor.matmul(s3_ps[:, 512:H],
                                 scoat1T[:, i * P:(i + 1) * P],
                                 bcoat[:, 512:H], start=True, stop=True)
                nc.scalar.copy(x_i[:, XO_S3:XO_S3 + H], s3_ps)
                ac_ps = bigp.tile([P, H], f32, tag="big768", name="big768")
                nc.tensor.matmul(ac_ps[:, 0:512],
                                 scoat1T[:, i * P:(i + 1) * P],
                                 qrows[:, 0:512], start=True, stop=True)
                nc.tensor.matmul(ac_ps[:, 512:H],
                                 scoat1T[:, i * P:(i + 1) * P],
                                 qrows[:, 512:H], start=True, stop=True)
                nc.scalar.copy(x_i[:, XO_AC:XO_AC + H], ac_ps)

                # layernorm
                stats = scr_pool.tile([P, 9, 6], f32, tag="stats", name="stats")
                xg = x_i.rearrange("p (g d) -> p g d", g=9)
                for g in range(9):
                    nc.vector.bn_stats(out=stats[:, g, :], in_=xg[:, g, :])
                mv = scr_pool.tile([P, 2], f32, tag="mv", name="mv")
                nc.vector.bn_aggr(out=mv, in_=stats)
                rsq = scr_pool.tile([P, 1], f32, tag="rsq", name="rsq")
                nc.scalar.activation(rsq, mv[:, 1:2], SQRT, bias=eps_sb,
                                     scale=1.0)
                rstd = scr_pool.tile([P, 1], f32, tag="rstd", name="rstd")
                nc.vector.reciprocal(rstd, rsq)
                negmr = scr_pool.tile([P, 1], f32, tag="negmr", name="negmr")
                nc.vector.tensor_scalar(negmr, mv[:, 0:1], rstd, -1.0,
                                        op0=MULT, op1=MULT)
                if trivial_gb:
                    # gamma==1, beta==0: y == normalized x, bf16 direct
                    nc.scalar.activation(
                        y_bf[i], x_i,
                        mybir.ActivationFunctionType.Identity,
                        bias=negmr, scale=rstd)
                    pending_y.append((i, y_bf[i]))
                else:
                    y_i = ysb_pool.tile([P, E2], f32, tag="y", name="y")
                    nc.scalar.activation(
                        y_i, x_i,
                        mybir.ActivationFunctionType.Identity,
                        bias=negmr, scale=rstd)
                    nc.vector.tensor_mul(y_i, y_i, gamma_b)
                    # full y (with beta) only needed by patt -> bf16
                    nc.vector.tensor_add(y_bf[i], y_i, beta_b)
                    pending_y.append((i, y_i))
                if i > 1:
                    pi, py = pending_y.pop(0)
                    _y8p_write(pi, py)
                if dbg:
                    nc.sync.dma_start(out=dbg["dbg_x"][i * P:(i + 1) * P, :],
                                      in_=x_i)
            for pi, py in pending_y:
                _y8p_write(pi, py)
        # stage-1 pools all freed

        # ================= phase 6: projections + scores + ss ========
        p56 = ExitStack()
        ssp = es.enter_context(tc.tile_pool(name="ssp", bufs=1))
        ss = [ssp.tile([P, CL], f32, tag=f"ss{i}", name=f"ss{i}")
              for i in range(CT_N)]
        with p56:
            prp = p56.enter_context(tc.tile_pool(name="prp", bufs=1))
            prps = p56.enter_context(
                tc.tile_pool(name="prps", bufs=3, space="PSUM"))
            scps = p56.enter_context(
                tc.tile_pool(name="scps", bufs=3, space="PSUM"))
            smp = p56.enter_context(tc.tile_pool(name="smp", bufs=4))

            bq2T = const.tile([P, ET], f32, tag="bq2T", name="bq2T")
            nc.sync.dma_start(out=bq2T,
                              in_=d_bq2.ap().rearrange("(t p) -> p t", p=P))
            bk2T = const.tile([P, ET], f32, tag="bk2T", name="bk2T")
            nc.sync.dma_start(out=bk2T,
                              in_=d_bk2.ap().rearrange("(t p) -> p t", p=P))
            IW = 1.0 / WSCALE
            for pair in range(NPAIR):
                e0 = pair * 384
                ops8 = {}
                for side, dw, bT in (("q", d_wq28, bq2T),
                                     ("k", d_wk28, bk2T)):
                    if pair == 0:
                        wt8 = w0[side]
                    else:
                        wt8 = wst.tile([P, ET2, 768], u8, tag="wchunk",
                                       name="wchunk")
                        src = dw.ap()[pair * ET2 * P:
                                      (pair + 1) * ET2 * P, :]
                        nc.sync.dma_start(
                            out=wt8,
                            in_=src.rearrange("(t p) f -> p t f", p=P))
                    wt4 = wt8.bitcast(f8).rearrange("p t (i m) -> p t i m",
                                                    i=2)
                    pss = []
                    for esub in range(3):
                        ps = prps.tile([P, CL], f32, tag="proj", name="proj")
                        for t in range(ET2):
                            nc.tensor.matmul(
                                ps, wt4[:, t, :, esub * P:(esub + 1) * P],
                                y8p[t], start=(t == 0), stop=(t == ET2 - 1),
                                perf_mode=DR)
                        pss.append(ps)
                    # repack the 384 head-pair dims into two fp8 [96, 2, CL]
                    # operands (per-head 192 = 96x2 contraction for one
                    # DoubleRow score matmul); bias add + 1/WSCALE rescale
                    # fused into the cast.
                    eb = e0 // P
                    b0 = bT[:, eb:eb + 1]
                    b1 = bT[:, eb + 1:eb + 2]
                    b2 = bT[:, eb + 2:eb + 3]
                    hA = prp.tile([96, 2, CL], f8, tag=f"hA{side}",
                                  name=f"hA{side}", bufs=2)
                    hB = prp.tile([96, 2, CL], f8, tag=f"hB{side}",
                                  name=f"hB{side}", bufs=2)
                    tsc = nc.vector.tensor_scalar
                    # partition-shifted repack in 32/64-aligned chunks
                    for dst, par, ofs, src_ps, slo, n, b in (
                            (hA, 0, 0, 0, 0, 64, b0),
                            (hA, 0, 64, 0, 64, 32, b0),
                            (hA, 1, 0, 0, 96, 32, b0),
                            (hA, 1, 32, 1, 0, 32, b1),
                            (hA, 1, 64, 1, 32, 32, b1),
                            (hB, 0, 0, 1, 64, 64, b1),
                            (hB, 0, 64, 2, 0, 32, b2),
                            (hB, 1, 0, 2, 32, 32, b2),
                            (hB, 1, 32, 2, 64, 32, b2),
                            (hB, 1, 64, 2, 96, 32, b2)):
                        tsc(dst[ofs:ofs + n, par, :],
                            pss[src_ps][slo:slo + n, :], b[slo:slo + n], IW,
                            op0=ADD, op1=MULT)
                    ops8[side] = (hA, hB)

                for hh in range(2):
                    head_idx = pair * 2 + hh
                    qh8 = ops8["q"][hh]
                    kh8 = ops8["k"][hh]
                    for i in range(CT_N):
                        ps = scps.tile([P, CL], f32, tag="sc2", name="sc2")
                        nc.tensor.matmul(ps, qh8[:, :, i * P:(i + 1) * P],
                                         kh8, start=True, stop=True,
                                         perf_mode=DR)
                        # unmasked softmax over keys, no max-subtract
                        # (|logit| smallish)
                        e_sb = smp.tile([P, CL], f32r, tag="e2",
                                        name="e2", bufs=2)
                        ssum = smp.tile([P, 1], f32, tag=f"ssum2_{i}",
                                        name=f"ssum2_{i}")
                        nc.scalar.activation(e_sb, ps, EXP, scale=ISQ,
                                             accum_out=ssum)
                        r = smp.tile([P, 1], f32, tag=f"r2_{i}",
                                     name=f"r2_{i}")
                        nc.vector.reciprocal(r, ssum)
                        r24 = smp.tile([P, 1], f32, tag=f"r242_{i}",
                                       name=f"r242_{i}")
                        nc.vector.tensor_scalar_mul(r24, r, 1.0 / NHEAD2)
                        # ss[i] += e * r24 on the vector engine (PE has no
                        # slack in stage 2; vector does)
                        if head_idx == 0:
                            nc.vector.tensor_scalar_mul(ss[i], e_sb, r24)
                        else:
                            nc.vector.scalar_tensor_tensor(
                                ss[i], in0=e_sb, scalar=r24,
                                in1=ss[i], op0=MULT, op1=ADD)

            if dbg:
                for i in range(CT_N):
                    nc.sync.dma_start(out=dbg["dbg_ss"][i * P:(i + 1) * P, :],
                                      in_=ss[i])
        # weight stream pools freed

        ytp_es.close()

        # ================= phase 7: ss1 + patt =================
        with ExitStack() as f7:
            fin = f7.enter_context(tc.tile_pool(name="fin", bufs=1))
            outp = f7.enter_context(tc.tile_pool(name="outp", bufs=3))
            pps = f7.enter_context(
                tc.tile_pool(name="pps", bufs=3, space="PSUM"))
            trp7 = f7.enter_context(
                tc.tile_pool(name="trp7", bufs=2, space="PSUM"))

            cm_b128 = const.tile([P, CL], f32, tag="cm_b128", name="cm_b128")
            nc.sync.dma_start(out=cm_b128, in_=d_cm.ap().partition_broadcast(P))
            ncm_b128 = const.tile([P, CL], f32, tag="ncm_b128", name="ncm_b128")
            nc.sync.dma_start(out=ncm_b128, in_=d_ncm.ap().partition_broadcast(P))

            ss1T = []
            for d in range(CT_N):
                sst = fin.tile([P, CL], f32, tag=f"ssT{d}", name=f"ssT{d}")
                for i in range(0, CT_N, 2):
                    str2 = trp7.tile([P, 2, P], f32, tag="tr", name="tr")
                    for k in range(2):
                        nc.tensor.transpose(
                            str2[:, k, :],
                            ss[i + k][:, d * P:(d + 1) * P], ident)
                    nc.vector.tensor_copy(sst[:, i * P:(i + 2) * P], str2)
                t = fin.tile([P, CL], bf16, tag=f"ss1T{d}", name=f"ss1T{d}")
                _masked_softmax(nc, fin, sst, t, cm_b128, ncm_b128, P, CL,
                                f"ss1_{d}")
                ss1T.append(t)

            for i in range(CT_N):
                x_i = x_re[i]
                for hs in range(E2 // 512):
                    ps = pps.tile([P, 512], f32, tag="patt", name="patt")
                    for d in range(CT_N):
                        nc.tensor.matmul(
                            ps, ss1T[d][:, i * P:(i + 1) * P],
                            y_bf[d][:, hs * 512:(hs + 1) * 512],
                            start=(d == 0), stop=(d == CT_N - 1))
                    o = outp.tile([P, 512], f32, tag="out", name="out")
                    nc.vector.tensor_add(o, ps,
                                         x_i[:, hs * 512:(hs + 1) * 512])
                    nc.sync.dma_start(
                        out=d_out[i * P:(i + 1) * P,
                                  hs * 512:(hs + 1) * 512],
                        in_=o)

    nc.compile()
    return nc


# ================= host side =================

_CACHE = {}


def _pack_w8(w2):
    """(E2, E2) float32 weight -> packed fp8 e4m3 (uint8 carrier).

    Layout [pair(12), t(18), p(128)] rows x [i(2), m(384)] cols, where
    w2t[k, e] lands at row (pair*18 + t)*128 + p, col i*384 + m with
    k = (2t + i)*128 + p, e = pair*384 + m. Scaled by WSCALE.
    """
    w2t = np.asarray(w2, np.float32).T
    w = (w2t * WSCALE).reshape(ET2, 2, P, NPAIR, 384)
    w8 = np.ascontiguousarray(w.transpose(3, 0, 2, 1, 4))
    q = w8.astype(ml_dtypes.float8_e4m3).view(np.uint8)
    return np.ascontiguousarray(q.reshape(NPAIR * ET2 * P, 768))


def prep_shared(inputs):
    f = np.float32
    cw2 = np.zeros((768, 2), f)
    cw2[:, 0] = np.asarray(inputs["c_weight"], f).reshape(-1)
    qw2 = np.zeros((768, 2), f)
    qw2[:, 0] = np.asarray(inputs["q_weight"], f).reshape(-1)
    return {
        "cw2": cw2,
        "qw2": qw2,
        "identity": np.eye(128, dtype=f),
        "cq_weight": np.ascontiguousarray(
            np.asarray(inputs["cq_weight"], f).reshape(-1)),
        "bias": np.ascontiguousarray(
            np.asarray(inputs["bias"], f).reshape(1, 1)),
        "wq1t": np.ascontiguousarray(np.asarray(inputs["wq1"], f).T),
        "wk1t": np.ascontiguousarray(np.asarray(inputs["wk1"], f).T),
        "bq1": np.ascontiguousarray(np.asarray(inputs["bq1"], f)),
        "bk1": np.ascontiguousarray(np.asarray(inputs["bk1"], f)),
        "gamma": np.ascontiguousarray(np.asarray(inputs["gamma"], f)),
        "beta": np.ascontiguousarray(np.asarray(inputs["beta"], f)),
        "wq28": _pack_w8(inputs["wq2"]),
        "wk28": _pack_w8(inputs["wk2"]),
        # beta is folded into the stage-2 projection bias: (y+beta)@W.T =
        # y@W.T + beta@W.T (the device projects the beta-less y)
        "bq2": np.ascontiguousarray(
            (np.asarray(inputs["bq2"], f)
             + np.asarray(inputs["wq2"], f) @ np.asarray(inputs["beta"], f))
            * WSCALE),
        "bk2": np.ascontiguousarray(
            (np.asarray(inputs["bk2"], f)
             + np.asarray(inputs["wk2"], f) @ np.asarray(inputs["beta"], f))
            * WSCALE),
    }


def make_in_maps(inputs, n_cores=8):
    f = np.float32
    shared = prep_shared(inputs)
    c = np.asarray(inputs["c"], f)
    q = np.asarray(inputs["q"], f)
    cm = np.asarray(inputs["c_mask"], f)
    qm = np.asarray(inputs["q_mask"], f)
    in_maps = []
    for b in range(n_cores):
        m = dict(shared)
        m["c"] = np.ascontiguousarray(c[b])
        m["q"] = np.ascontiguousarray(q[b])
        m["cT"] = np.ascontiguousarray(c[b].T)
        m["qT"] = np.ascontiguousarray(q[b].T)
        m["cm"] = np.ascontiguousarray(cm[b])
        m["ncm"] = np.ascontiguousarray((1.0 - cm[b]) * np.float32(NEG))
        m["qm"] = np.ascontiguousarray(qm[b])
        m["nqm"] = np.ascontiguousarray((1.0 - qm[b]) * np.float32(NEG))
        in_maps.append(m)
    return in_maps


def kernel(**inputs):
    from concourse.bass_utils import run_bass_kernel_spmd

    B = inputs["c"].shape[0]
    trivial_gb = bool(
        np.all(np.asarray(inputs["gamma"], np.float32) == 1.0)
        and np.all(np.asarray(inputs["beta"], np.float32) == 0.0))
    key = ("nc", trivial_gb)
    if key not in _CACHE:
        _CACHE[key] = build(num_devices=B, trivial_gb=trivial_gb)
    nc = _CACHE[key]
    in_maps = make_in_maps(inputs, B)
    res = run_bass_kernel_spmd(nc, in_maps, core_ids=list(range(B)))
    out = np.stack([res.results[b]["out"] for b in range(B)])
    return out
